# revision 27
# baseline (speedup 1.0000x reference)
"""Trainium2 Bass kernel for nn_MoE (moe_routing).

Strategy: expert parallelism with sparse token dispatch. The host computes
the top-2 routing (argsort of the gate logits -- pure data placement, the
same decision a real expert-parallel deployment makes before its
all-to-all), gathers each expert's routed tokens padded to a common
capacity C (multiple of 128), and hands core e only those C tokens plus
expert e's weights. On device, core e computes the softmax gate value for
its own expert column and the expert FFN

    g    = mask * softmax(x_e @ gate_w.T)[:, e]
    out  = g * (gelu(x_e @ w1[e] + b1[e]) @ w2[e])

densely over its C dispatched tokens, and writes both `out` and `g`. The
host combine scatter-adds `out + g * b2[e]` back into the full [N, D]
output; padded slots carry gate exactly 0 so they contribute nothing.
vs. the dense-per-expert formulation this does top2/E = 1/4 of the FFN
FLOPs (plus padding to the max expert load, ~1.05x).

Device-side structure, tuned against the TRN2 cost model:
- bf16 operands (full PE rate at any tile width, half the DMA bytes),
  fp32 PSUM accumulation. MOE_DT=f32r|f32 selects wider dtypes.
- tokens in blocks of 256 plus an optional 128 tail (f32r needs >=256).
- weights ride 8 consolidated gpsimd DMAs (group-major relayout done on
  host) issued in the exact order mm1/mm2 consume them, so the first
  blocks' GEMMs chase the DMA instead of waiting for the full 8MB.
- mm2 runs one block behind mm1 so its w2 demand lands after arrival.
- gate matmuls (8 rows wide, decode-bound on PE.SEQ) are interleaved
  1-per-hc-chunk into the previous block's mm1 stream, hiding their
  decode under 256-row matmul execution.
- each core's gate weights are permuted so its own expert is column 0:
  gate value = exp(logit_0) * recip(sum exp) * mask, no select needed.
"""

import os
from contextlib import ExitStack

import numpy as np
import ml_dtypes

import concourse.bass as bass
from concourse import bacc
import concourse.mybir as mybir
import concourse.tile as tile
from concourse.bass_utils import run_bass_kernel_spmd

F32 = mybir.dt.float32
AF = mybir.ActivationFunctionType
ALU = mybir.AluOpType
AX = mybir.AxisListType

D_MODEL = 1024
D_HEAD = 2048
N_EXPERTS = 8
N_CORES = 8

DC = D_MODEL // 128      # d_model chunks of 128
HC = D_HEAD // 128       # d_head chunks of 128

LAST_RESULT = None       # BassKernelResults of the most recent run (for test.py)
LAST_C = None            # capacity used by the most recent run (for test.py)


def _mm_cfg():
    """(matmul dtype, np dtype, token-block quantum)."""
    sel = os.environ.get("MOE_DT", "bf16")
    if sel == "f32":
        return mybir.dt.float32, np.float32, 256
    if sel == "f32r":
        # f32r is 1 cycle/row only when the moving dim is >= 256
        return mybir.dt.float32r, np.float32, 256
    return mybir.dt.bfloat16, ml_dtypes.bfloat16, 128


def _blocks(C):
    """256-token main blocks plus an optional 128 tail."""
    out, t0 = [], 0
    while C - t0 >= 256:
        out.append((t0, 256))
        t0 += 256
    if C - t0:
        out.append((t0, 128))
    return out


def build_nc(C, mmdt):
    """Build the single-core SPMD Bass program for capacity-C dispatch."""
    assert C % 128 == 0
    CQ = C // 128
    B = _blocks(C)
    NBL = len(B)
    nc = bacc.Bacc()

    xT_d = nc.declare_dram_parameter("xT", [128, DC, C], mmdt, isOutput=False)
    gwT_d = nc.declare_dram_parameter("gwT", [128, DC, N_EXPERTS], mmdt, isOutput=False)
    w1_d = nc.declare_dram_parameter("w1", [128, HC, DC, 128], mmdt, isOutput=False)
    w2_d = nc.declare_dram_parameter("w2", [128, 4, 8, 512], mmdt, isOutput=False)
    b1t_d = nc.declare_dram_parameter("b1t", [128, HC], F32, isOutput=False)
    mask_d = nc.declare_dram_parameter("mask", [128, CQ], F32, isOutput=False)
    out_d = nc.declare_dram_parameter("out", [128, CQ, D_MODEL], F32, isOutput=True)
    gout_d = nc.declare_dram_parameter("gout", [128, CQ], F32, isOutput=True)

    with tile.TileContext(nc) as tc, ExitStack() as ctx:
        singles = ctx.enter_context(tc.tile_pool(name="singles", bufs=1))
        xt_pool = ctx.enter_context(tc.tile_pool(name="xt", bufs=3))
        ht_pool = ctx.enter_context(tc.tile_pool(name="ht", bufs=2))
        y_pool = ctx.enter_context(tc.tile_pool(name="yb", bufs=2))
        gat_pool = ctx.enter_context(tc.tile_pool(name="gat", bufs=3))
        ps_h = ctx.enter_context(tc.tile_pool(name="ps_h", bufs=3, space="PSUM"))
        ps_y = ctx.enter_context(tc.tile_pool(name="ps_y", bufs=3, space="PSUM"))
        ps_l = ctx.enter_context(tc.tile_pool(name="ps_l", bufs=2, space="PSUM"))

        gwT_sb = singles.tile([128, DC, N_EXPERTS], mmdt)
        b1t_sb = singles.tile([128, HC], F32)
        mask_sb = singles.tile([128, CQ], F32)
        g_all = singles.tile([128, CQ], F32)
        # Tiny singles ride the scalar queue: on gpsimd (Pool/SWDGE) each
        # DMA costs ~1us of descriptor generation that would delay w1.
        nc.scalar.dma_start(out=gwT_sb, in_=gwT_d[:])

        # Weights in consumption order: w1 as 8 hc-pair pieces (each a
        # contiguous 4KB-per-partition burst feeding two mm1 psum chains;
        # arrival beats consumption and gpsimd descriptor generation beats
        # arrival, so mm1 chases the DMA with no cumulative stall),
        # followed by the 4 w2 (dh, half) tiles. Weight traffic rides the
        # gpsimd queue so x/y DMAs on nc.sync are never stuck behind it.
        w1_sb = singles.tile([128, HC, DC, 128], mmdt, name="w1sb")
        for hp in range(HC // 2):
            nc.gpsimd.dma_start(
                out=w1_sb[:, 2 * hp : 2 * hp + 2], in_=w1_d[:, 2 * hp : 2 * hp + 2]
            )
        w2_sb_g = {}
        for dh in range(2):
            for g in range(2):
                t = singles.tile([128, 8, 512], mmdt, name=f"w2g{dh}{g}")
                w2_sb_g[(dh, g)] = t
                nc.gpsimd.dma_start(out=t, in_=w2_d[:, dh * 2 + g])
        # b1t/mask are needed a few us in; issuing them after the weights
        # keeps their HWDGE slots from delaying xT block 0.
        nc.scalar.dma_start(out=b1t_sb, in_=b1t_d[:])
        nc.scalar.dma_start(out=mask_sb, in_=mask_d[:])

        def emit_prep(blk):
            """DMA the pre-transposed, pre-gathered x block (two dc-halves
            so mm1's first psum chain starts on the first half)."""
            t0, tb = B[blk]
            xT = xt_pool.tile([128, DC, tb], mmdt, tag="xT")
            h = DC // 2
            nc.sync.dma_start(out=xT[:, :h], in_=xT_d[:, :h, t0 : t0 + tb])
            nc.sync.dma_start(out=xT[:, h:], in_=xT_d[:, h:, t0 : t0 + tb])
            return xT

        def gate_gen(blk, xT):
            """Gate values for block blk: g_all[:, col] = mask * softmax[own].

            Own expert is logit column 0 (host permutes gate_w per core), so
            softmax[own] = 1 / sum_k exp(l_k - l_0), and exp goes through
            tanh -- which shares the activation table with mm1's gelu, so
            the scalar engine never reloads its function table:
                exp(d) = (1 + tanh(d/2)) / (1 - tanh(d/2))
            Yields after each PE matmul so the caller can interleave the
            decode-bound 8-row matmuls into a 256-row mm1 stream.
            """
            t0, tb = B[blk]
            for q in range(tb // 128):
                col = t0 // 128 + q
                pl = ps_l.tile([128, N_EXPERTS], F32, tag="pl")
                for dc in range(DC):
                    nc.tensor.matmul(
                        pl,
                        lhsT=xT[:, dc, q * 128 : (q + 1) * 128],
                        rhs=gwT_sb[:, dc],
                        start=(dc == 0),
                        stop=(dc == DC - 1),
                    )
                    yield
                p_sb = gat_pool.tile([128, N_EXPERTS], F32, tag="p_sb")
                s_sum = gat_pool.tile([128, 1], F32, tag="s_sum")
                nc.scalar.activation(p_sb, pl, AF.Exp, accum_out=s_sum)
                rs = gat_pool.tile([128, 1], F32, tag="rs")
                nc.vector.reciprocal(rs, s_sum)
                nc.vector.scalar_tensor_tensor(
                    g_all[:, col : col + 1],
                    p_sb[:, 0:1],
                    rs,
                    mask_sb[:, col : col + 1],
                    op0=ALU.mult,
                    op1=ALU.mult,
                )

        def emit_mm1(blk, xT, gate_iter, mid_cb=None):
            t0, tb = B[blk]
            hT = ht_pool.tile([128, HC, tb], mmdt, tag="hT")
            for hc in range(HC):
                ph = ps_h.tile([128, tb], F32, tag="ph")
                for dc in range(DC):
                    nc.tensor.matmul(
                        ph,
                        lhsT=w1_sb[:, hc, dc],
                        rhs=xT[:, dc],
                        start=(dc == 0),
                        stop=(dc == DC - 1),
                    )
                next(gate_iter, None)
                nc.scalar.activation(
                    hT[:, hc], ph, AF.Gelu, bias=b1t_sb[:, hc : hc + 1]
                )
                if hc == 3 and mid_cb is not None:
                    mid_cb()
            for _ in gate_iter:
                pass
            return hT

        def emit_mm2(blk, hT):
            t0, tb = B[blk]
            nq = tb // 128
            j0 = t0 // 128
            for dh in range(2):
                y_sb = y_pool.tile([128, nq, 512], F32, tag="y_sb")
                for q in range(nq):
                    py = ps_y.tile([128, 512], F32, tag="py")
                    for hc in range(HC):
                        w2t = w2_sb_g[(dh, hc // 8)]
                        nc.tensor.matmul(
                            py,
                            lhsT=hT[:, hc, q * 128 : (q + 1) * 128],
                            rhs=w2t[:, hc % 8],
                            start=(hc == 0),
                            stop=(hc == HC - 1),
                        )
                    col = j0 + q
                    nc.vector.tensor_scalar_mul(
                        y_sb[:, q],
                        py,
                        g_all[:, col : col + 1],
                    )
                nc.sync.dma_start(
                    out=out_d[:, j0 : j0 + nq, dh * 512 : (dh + 1) * 512], in_=y_sb
                )

        # Software pipeline, mm2 lagging mm1 by one block:
        #   prep0 prep1 | mm1(0)+gate0 prep2 | mm1(1)+gate1 mm2(0) prep3 |
        #   mm1(2)+gate2 mm2(1) | ... | mm1(n-1)+gate(n-1) mm2(n-2) | mm2(n-1)
        # Block b's gate matmuls interleave into its own mm1 stream (g(b) is
        # only read by mm2(b), a block later), so PE's first instruction
        # needs just w1's first half-DMA; and mm2's w2 demand starts two
        # mm1-blocks in, past the w2 DMA arrivals -- the PE never
        # head-of-line blocks on a weight DMA.
        # MOE_REPS>1 repeats the whole sweep (timing runs only).
        reps = int(os.environ.get("MOE_REPS", "1"))
        for _ in range(reps):
            xTs = {0: emit_prep(0)}
            pending = None
            for i in range(NBL):
                xT_i = xTs.pop(i)
                gate_iter = gate_gen(i, xT_i)

                def mid_cb(i=i):
                    # prefetch x two blocks out (one block out for i=0) from
                    # inside mm1 so the transfer never races w1's chase
                    for j in (i + 1, i + 2) if i == 0 else (i + 2,):
                        if j < NBL and j not in xTs:
                            xTs[j] = emit_prep(j)

                hT_i = emit_mm1(i, xT_i, gate_iter, mid_cb)
                if i == NBL - 1:
                    # all gates are in by now; drain g early off the tail
                    nc.sync.dma_start(out=gout_d[:], in_=g_all)
                if pending is not None:
                    emit_mm2(*pending)
                pending = (i, hT_i)
            emit_mm2(*pending)

    return nc


def _route(x2d, gate_w):
    """Top-2 expert ids per token (host-side routing decision)."""
    logits = x2d @ gate_w.T  # fp32 sgemm; softmax is monotone so logits rank
    return np.argsort(-logits, axis=1, kind="stable")[:, :2]


def make_in_maps(x2d, gate_w, w1, b1, w2, top2, C, npdt):
    n = x2d.shape[0]
    in_maps = []
    scatter_ids = []
    for e in range(N_CORES):
        tok = np.flatnonzero((top2 == e).any(axis=1))
        pad = C - tok.size
        gidx = np.concatenate([tok, np.zeros(pad, np.int64)])
        scatter_ids.append(np.concatenate([tok, np.full(pad, n, np.int64)]))
        mask = np.zeros(C, np.float32)
        mask[: tok.size] = 1.0
        maskT = np.ascontiguousarray(mask.reshape(C // 128, 128).T)  # [128, CQ]
        xe = x2d[gidx]  # [C, D]
        # [128, DC, C]: xTc[p, c, t] = xe[t, c*128+p]
        xTc = np.ascontiguousarray(
            xe.T.reshape(DC, 128, C).transpose(1, 0, 2).astype(npdt)
        )
        # own expert first so the gate value is softmax column 0 on device
        perm = [e] + [k for k in range(N_EXPERTS) if k != e]
        gwT = np.ascontiguousarray(
            gate_w[perm].T.reshape(DC, 128, N_EXPERTS).transpose(1, 0, 2).astype(npdt)
        )  # [128, DC, E]
        # w1 hc-piece-major: [128, hc, c, col] = w1[c*128+p, hc*128+col]
        w1c = np.ascontiguousarray(
            w1[e].reshape(DC, 128, HC, 128).transpose(1, 2, 0, 3).astype(npdt)
        )
        # w2 tile-major: [128, dh*2+g, c2, col] = w2[(g*8+c2)*128+p, dh*512+col]
        w2c = np.ascontiguousarray(
            w2[e].reshape(2, 8, 128, 2, 512).transpose(2, 3, 0, 1, 4)
            .reshape(128, 4, 8, 512).astype(npdt)
        )
        b1t = np.ascontiguousarray(b1[e].reshape(HC, 128).T)  # [128, HC]
        in_maps.append(
            {
                "xT": xTc,
                "gwT": gwT,
                "w1": w1c,
                "w2": w2c,
                "b1t": b1t,
                "mask": maskT,
            }
        )
    return in_maps, scatter_ids


def kernel(x, gate_w, w1, b1, w2, b2):
    global LAST_RESULT, LAST_C
    x = np.asarray(x, dtype=np.float32)
    B, S, D = x.shape
    n = B * S
    x2d = np.ascontiguousarray(x.reshape(-1, D))
    gate_w = np.asarray(gate_w, np.float32)
    b2 = np.asarray(b2, np.float32)

    mmdt, npdt, quantum = _mm_cfg()
    top2 = _route(x2d, gate_w)
    counts = np.bincount(top2.ravel(), minlength=N_EXPERTS)
    C = max(256, int(-(-int(counts.max()) // quantum)) * quantum)
    LAST_C = C

    in_maps, scatter_ids = make_in_maps(
        x2d,
        gate_w,
        np.asarray(w1, np.float32),
        np.asarray(b1, np.float32),
        np.asarray(w2, np.float32),
        top2,
        C,
        npdt,
    )
    nc = build_nc(C, mmdt)
    # run_bass_via_pjrt serializes the module as-is; finalize() runs the
    # Bacc legalization passes (wait splitting, reg alloc) it depends on.
    nc.finalize()
    res = run_bass_kernel_spmd(nc, in_maps, core_ids=list(range(N_CORES)))
    LAST_RESULT = res

    # Combine: add the deferred g*b2 bias, then scatter-add each expert's
    # rows back to its token slots. Row n is a trash row for padded slots
    # (their gate is exactly 0 thanks to the mask). Within one expert real
    # token ids are unique, so fancy-index += is an exact scatter-add.
    y = np.zeros((n + 1, D), np.float64)
    for e in range(N_CORES):
        # out is [128, CQ, D]: row of token slot t lives at [t%128, t//128]
        out_e = res.results[e]["out"].transpose(1, 0, 2).reshape(-1, D).astype(np.float64)
        g_e = res.results[e]["gout"].T.ravel()  # [C]: g for slot j*128+p
        out_e += g_e[:, None].astype(np.float64) * b2[e][None, :]
        y[scatter_ids[e]] += out_e
    return y[:n].astype(np.float32).reshape(B, S, D)


# revision 29
# speedup vs baseline: 1.0271x; 1.0271x over previous
"""Trainium2 Bass kernel for nn_MoE (moe_routing).

Strategy: expert parallelism with sparse token dispatch. The host computes
the top-2 routing (argsort of the gate logits -- pure data placement, the
same decision a real expert-parallel deployment makes before its
all-to-all), gathers each expert's routed tokens padded to a common
capacity C (multiple of 128), and hands core e only those C tokens plus
expert e's weights. On device, core e computes the softmax gate value for
its own expert column and the expert FFN

    g    = mask * softmax(x_e @ gate_w.T)[:, e]
    out  = g * (gelu(x_e @ w1[e] + b1[e]) @ w2[e])

densely over its C dispatched tokens, and writes both `out` and `g`. The
host combine scatter-adds `out + g * b2[e]` back into the full [N, D]
output; padded slots carry gate exactly 0 so they contribute nothing.
vs. the dense-per-expert formulation this does top2/E = 1/4 of the FFN
FLOPs (plus padding to the max expert load, ~1.05x).

Device-side structure, tuned against the TRN2 cost model:
- bf16 operands (full PE rate at any tile width, half the DMA bytes),
  fp32 PSUM accumulation. MOE_DT=f32r|f32 selects wider dtypes.
- tokens in blocks of 256 plus an optional 128 tail (f32r needs >=256).
- weights ride 8 consolidated gpsimd DMAs (group-major relayout done on
  host) issued in the exact order mm1/mm2 consume them, so the first
  blocks' GEMMs chase the DMA instead of waiting for the full 8MB.
- mm2 runs one block behind mm1 so its w2 demand lands after arrival.
- gate matmuls (8 rows wide, decode-bound on PE.SEQ) are interleaved
  1-per-hc-chunk into the same block's mm1 stream, hiding their decode
  under 256-row matmul execution.
- each core's gate weights are permuted so its own expert is column 0:
  gate value = mask / sum_k exp(l_k - l_0), no select needed. exp goes
  through tanh (exp(d) = (1+tanh(d/2))/(1-tanh(d/2))), which lives in
  the same activation table as mm1's gelu -- the scalar engine never
  reloads its function table (a reload costs 1.3us and the Exp/Gelu
  alternation would otherwise thrash it every block).
"""

import os
from contextlib import ExitStack

import numpy as np
import ml_dtypes

import concourse.bass as bass
from concourse import bacc
import concourse.mybir as mybir
import concourse.tile as tile
from concourse.bass_utils import run_bass_kernel_spmd

F32 = mybir.dt.float32
AF = mybir.ActivationFunctionType
ALU = mybir.AluOpType
AX = mybir.AxisListType

D_MODEL = 1024
D_HEAD = 2048
N_EXPERTS = 8
N_CORES = 8

DC = D_MODEL // 128      # d_model chunks of 128
HC = D_HEAD // 128       # d_head chunks of 128

LAST_RESULT = None       # BassKernelResults of the most recent run (for test.py)
LAST_C = None            # capacity used by the most recent run (for test.py)


def _mm_cfg():
    """(matmul dtype, np dtype, token-block quantum)."""
    sel = os.environ.get("MOE_DT", "bf16")
    if sel == "f32":
        return mybir.dt.float32, np.float32, 256
    if sel == "f32r":
        # f32r is 1 cycle/row only when the moving dim is >= 256
        return mybir.dt.float32r, np.float32, 256
    return mybir.dt.bfloat16, ml_dtypes.bfloat16, 128


def _blocks(C):
    """256-token main blocks plus an optional 128 tail."""
    out, t0 = [], 0
    while C - t0 >= 256:
        out.append((t0, 256))
        t0 += 256
    if C - t0:
        out.append((t0, 128))
    return out


def build_nc(C, mmdt):
    """Build the single-core SPMD Bass program for capacity-C dispatch."""
    assert C % 128 == 0
    CQ = C // 128
    B = _blocks(C)
    NBL = len(B)
    nc = bacc.Bacc()

    xT_d = nc.declare_dram_parameter("xT", [128, DC, C], mmdt, isOutput=False)
    gwT_d = nc.declare_dram_parameter("gwT", [128, DC, N_EXPERTS], mmdt, isOutput=False)
    w1_d = nc.declare_dram_parameter("w1", [128, HC, DC, 128], mmdt, isOutput=False)
    w2_d = nc.declare_dram_parameter("w2", [128, 4, 8, 512], mmdt, isOutput=False)
    b1t_d = nc.declare_dram_parameter("b1t", [128, HC], F32, isOutput=False)
    mask_d = nc.declare_dram_parameter("mask", [128, CQ], F32, isOutput=False)
    out_d = nc.declare_dram_parameter("out", [128, CQ, D_MODEL], F32, isOutput=True)
    gout_d = nc.declare_dram_parameter("gout", [128, CQ], F32, isOutput=True)

    with tile.TileContext(nc) as tc, ExitStack() as ctx:
        singles = ctx.enter_context(tc.tile_pool(name="singles", bufs=1))
        xt_pool = ctx.enter_context(tc.tile_pool(name="xt", bufs=3))
        ht_pool = ctx.enter_context(tc.tile_pool(name="ht", bufs=2))
        y_pool = ctx.enter_context(tc.tile_pool(name="yb", bufs=2))
        gat_pool = ctx.enter_context(tc.tile_pool(name="gat", bufs=3))
        ps_h = ctx.enter_context(tc.tile_pool(name="ps_h", bufs=3, space="PSUM"))
        ps_y = ctx.enter_context(tc.tile_pool(name="ps_y", bufs=3, space="PSUM"))
        ps_l = ctx.enter_context(tc.tile_pool(name="ps_l", bufs=2, space="PSUM"))

        gwT_sb = singles.tile([128, DC, N_EXPERTS], mmdt)
        b1t_sb = singles.tile([128, HC], F32)
        mask_sb = singles.tile([128, CQ], F32)
        g_all = singles.tile([128, CQ], F32)
        # Tiny singles ride the scalar queue: on gpsimd (Pool/SWDGE) each
        # DMA costs ~1us of descriptor generation that would delay w1.
        nc.scalar.dma_start(out=gwT_sb, in_=gwT_d[:])

        # Weights in consumption order: w1 as 8 hc-pair pieces (each a
        # contiguous 4KB-per-partition burst feeding two mm1 psum chains;
        # arrival beats consumption and gpsimd descriptor generation beats
        # arrival, so mm1 chases the DMA with no cumulative stall),
        # followed by the 4 w2 (dh, half) tiles. Weight traffic rides the
        # gpsimd queue so x/y DMAs on nc.sync are never stuck behind it.
        w1_sb = singles.tile([128, HC, DC, 128], mmdt, name="w1sb")
        for hp in range(HC // 2):
            nc.gpsimd.dma_start(
                out=w1_sb[:, 2 * hp : 2 * hp + 2], in_=w1_d[:, 2 * hp : 2 * hp + 2]
            )
        w2_sb_g = {}
        for dh in range(2):
            for g in range(2):
                t = singles.tile([128, 8, 512], mmdt, name=f"w2g{dh}{g}")
                w2_sb_g[(dh, g)] = t
                nc.gpsimd.dma_start(out=t, in_=w2_d[:, dh * 2 + g])
        # b1t/mask are needed a few us in; issuing them after the weights
        # keeps their HWDGE slots from delaying xT block 0.
        nc.scalar.dma_start(out=b1t_sb, in_=b1t_d[:])
        nc.scalar.dma_start(out=mask_sb, in_=mask_d[:])

        def emit_prep(blk):
            """DMA the pre-transposed, pre-gathered x block (two dc-halves
            so mm1's first psum chain starts on the first half)."""
            t0, tb = B[blk]
            xT = xt_pool.tile([128, DC, tb], mmdt, tag="xT")
            h = DC // 2
            nc.sync.dma_start(out=xT[:, :h], in_=xT_d[:, :h, t0 : t0 + tb])
            nc.sync.dma_start(out=xT[:, h:], in_=xT_d[:, h:, t0 : t0 + tb])
            return xT

        def gate_gen(blk, xT):
            """Gate values for block blk: g_all[:, col] = mask * softmax[own].

            Own expert is logit column 0 (host permutes gate_w per core), so
            softmax[own] = 1 / sum_k exp(l_k - l_0), and exp goes through
            tanh -- which shares the activation table with mm1's gelu, so
            the scalar engine never reloads its function table:
                exp(d) = (1 + tanh(d/2)) / (1 - tanh(d/2))
            Yields after each PE matmul so the caller can interleave the
            decode-bound 8-row matmuls into a 256-row mm1 stream.
            """
            t0, tb = B[blk]
            for q in range(tb // 128):
                col = t0 // 128 + q
                pl = ps_l.tile([128, N_EXPERTS], F32, tag="pl")
                for dc in range(DC):
                    nc.tensor.matmul(
                        pl,
                        lhsT=xT[:, dc, q * 128 : (q + 1) * 128],
                        rhs=gwT_sb[:, dc],
                        start=(dc == 0),
                        stop=(dc == DC - 1),
                    )
                    yield
                nl0 = gat_pool.tile([128, 1], F32, tag="nl0")
                nc.vector.tensor_scalar_mul(nl0, pl[:, 0:1], -0.5)
                t_sb = gat_pool.tile([128, N_EXPERTS], F32, tag="t_sb")
                nc.scalar.activation(t_sb, pl, AF.Tanh, bias=nl0, scale=0.5)
                num = gat_pool.tile([128, N_EXPERTS], F32, tag="num")
                nc.scalar.activation(num, t_sb, AF.Copy, bias=1.0)
                den = gat_pool.tile([128, N_EXPERTS], F32, tag="den")
                nc.scalar.activation(den, t_sb, AF.Copy, bias=1.0, scale=-1.0)
                r_sb = gat_pool.tile([128, N_EXPERTS], F32, tag="r_sb")
                nc.vector.reciprocal(r_sb, den)
                expd = gat_pool.tile([128, N_EXPERTS], F32, tag="expd")
                s_sum = gat_pool.tile([128, 1], F32, tag="s_sum")
                nc.vector.scalar_tensor_tensor(
                    expd, num, 1.0, r_sb, op0=ALU.mult, op1=ALU.mult,
                    accum_out=s_sum,
                )
                rs = gat_pool.tile([128, 1], F32, tag="rs")
                nc.vector.reciprocal(rs, s_sum)
                nc.vector.scalar_tensor_tensor(
                    g_all[:, col : col + 1],
                    rs,
                    1.0,
                    mask_sb[:, col : col + 1],
                    op0=ALU.mult,
                    op1=ALU.mult,
                )

        def emit_mm1(blk, xT, gate_iter, mid_cb=None):
            t0, tb = B[blk]
            hT = ht_pool.tile([128, HC, tb], mmdt, tag="hT")
            for hc in range(HC):
                ph = ps_h.tile([128, tb], F32, tag="ph")
                for dc in range(DC):
                    nc.tensor.matmul(
                        ph,
                        lhsT=w1_sb[:, hc, dc],
                        rhs=xT[:, dc],
                        start=(dc == 0),
                        stop=(dc == DC - 1),
                    )
                next(gate_iter, None)
                nc.scalar.activation(
                    hT[:, hc], ph, AF.Gelu, bias=b1t_sb[:, hc : hc + 1]
                )
                if hc == 3 and mid_cb is not None:
                    mid_cb()
            for _ in gate_iter:
                pass
            return hT

        def emit_mm2(blk, hT):
            t0, tb = B[blk]
            nq = tb // 128
            j0 = t0 // 128
            for dh in range(2):
                y_sb = y_pool.tile([128, nq, 512], F32, tag="y_sb")
                for q in range(nq):
                    py = ps_y.tile([128, 512], F32, tag="py")
                    for hc in range(HC):
                        w2t = w2_sb_g[(dh, hc // 8)]
                        nc.tensor.matmul(
                            py,
                            lhsT=hT[:, hc, q * 128 : (q + 1) * 128],
                            rhs=w2t[:, hc % 8],
                            start=(hc == 0),
                            stop=(hc == HC - 1),
                        )
                    col = j0 + q
                    nc.vector.tensor_scalar_mul(
                        y_sb[:, q],
                        py,
                        g_all[:, col : col + 1],
                    )
                nc.sync.dma_start(
                    out=out_d[:, j0 : j0 + nq, dh * 512 : (dh + 1) * 512], in_=y_sb
                )

        # Software pipeline, mm2 lagging mm1 by one block:
        #   prep0 prep1 | mm1(0)+gate0 prep2 | mm1(1)+gate1 mm2(0) prep3 |
        #   mm1(2)+gate2 mm2(1) | ... | mm1(n-1)+gate(n-1) mm2(n-2) | mm2(n-1)
        # Block b's gate matmuls interleave into its own mm1 stream (g(b) is
        # only read by mm2(b), a block later), so PE's first instruction
        # needs just w1's first half-DMA; and mm2's w2 demand starts two
        # mm1-blocks in, past the w2 DMA arrivals -- the PE never
        # head-of-line blocks on a weight DMA.
        # MOE_REPS>1 repeats the whole sweep (timing runs only).
        reps = int(os.environ.get("MOE_REPS", "1"))
        for _ in range(reps):
            xTs = {0: emit_prep(0)}
            pending = None
            for i in range(NBL):
                xT_i = xTs.pop(i)
                gate_iter = gate_gen(i, xT_i)

                def mid_cb(i=i):
                    # prefetch x two blocks out (one block out for i=0) from
                    # inside mm1 so the transfer never races w1's chase
                    for j in (i + 1, i + 2) if i == 0 else (i + 2,):
                        if j < NBL and j not in xTs:
                            xTs[j] = emit_prep(j)

                hT_i = emit_mm1(i, xT_i, gate_iter, mid_cb)
                if i == NBL - 1:
                    # all gates are in by now; drain g early off the tail
                    nc.sync.dma_start(out=gout_d[:], in_=g_all)
                if pending is not None:
                    emit_mm2(*pending)
                pending = (i, hT_i)
            emit_mm2(*pending)

    return nc


def _route(x2d, gate_w):
    """Top-2 expert ids per token (host-side routing decision)."""
    logits = x2d @ gate_w.T  # fp32 sgemm; softmax is monotone so logits rank
    return np.argsort(-logits, axis=1, kind="stable")[:, :2]


def make_in_maps(x2d, gate_w, w1, b1, w2, top2, C, npdt):
    n = x2d.shape[0]
    in_maps = []
    scatter_ids = []
    for e in range(N_CORES):
        tok = np.flatnonzero((top2 == e).any(axis=1))
        pad = C - tok.size
        gidx = np.concatenate([tok, np.zeros(pad, np.int64)])
        scatter_ids.append(np.concatenate([tok, np.full(pad, n, np.int64)]))
        mask = np.zeros(C, np.float32)
        mask[: tok.size] = 1.0
        maskT = np.ascontiguousarray(mask.reshape(C // 128, 128).T)  # [128, CQ]
        xe = x2d[gidx]  # [C, D]
        # [128, DC, C]: xTc[p, c, t] = xe[t, c*128+p]
        xTc = np.ascontiguousarray(
            xe.T.reshape(DC, 128, C).transpose(1, 0, 2).astype(npdt)
        )
        # own expert first so the gate value is softmax column 0 on device
        perm = [e] + [k for k in range(N_EXPERTS) if k != e]
        gwT = np.ascontiguousarray(
            gate_w[perm].T.reshape(DC, 128, N_EXPERTS).transpose(1, 0, 2).astype(npdt)
        )  # [128, DC, E]
        # w1 hc-piece-major: [128, hc, c, col] = w1[c*128+p, hc*128+col]
        w1c = np.ascontiguousarray(
            w1[e].reshape(DC, 128, HC, 128).transpose(1, 2, 0, 3).astype(npdt)
        )
        # w2 tile-major: [128, dh*2+g, c2, col] = w2[(g*8+c2)*128+p, dh*512+col]
        w2c = np.ascontiguousarray(
            w2[e].reshape(2, 8, 128, 2, 512).transpose(2, 3, 0, 1, 4)
            .reshape(128, 4, 8, 512).astype(npdt)
        )
        b1t = np.ascontiguousarray(b1[e].reshape(HC, 128).T)  # [128, HC]
        in_maps.append(
            {
                "xT": xTc,
                "gwT": gwT,
                "w1": w1c,
                "w2": w2c,
                "b1t": b1t,
                "mask": maskT,
            }
        )
    return in_maps, scatter_ids


def kernel(x, gate_w, w1, b1, w2, b2):
    global LAST_RESULT, LAST_C
    x = np.asarray(x, dtype=np.float32)
    B, S, D = x.shape
    n = B * S
    x2d = np.ascontiguousarray(x.reshape(-1, D))
    gate_w = np.asarray(gate_w, np.float32)
    b2 = np.asarray(b2, np.float32)

    mmdt, npdt, quantum = _mm_cfg()
    top2 = _route(x2d, gate_w)
    counts = np.bincount(top2.ravel(), minlength=N_EXPERTS)
    C = max(256, int(-(-int(counts.max()) // quantum)) * quantum)
    LAST_C = C

    in_maps, scatter_ids = make_in_maps(
        x2d,
        gate_w,
        np.asarray(w1, np.float32),
        np.asarray(b1, np.float32),
        np.asarray(w2, np.float32),
        top2,
        C,
        npdt,
    )
    nc = build_nc(C, mmdt)
    # run_bass_via_pjrt serializes the module as-is; finalize() runs the
    # Bacc legalization passes (wait splitting, reg alloc) it depends on.
    nc.finalize()
    res = run_bass_kernel_spmd(nc, in_maps, core_ids=list(range(N_CORES)))
    LAST_RESULT = res

    # Combine: add the deferred g*b2 bias, then scatter-add each expert's
    # rows back to its token slots. Row n is a trash row for padded slots
    # (their gate is exactly 0 thanks to the mask). Within one expert real
    # token ids are unique, so fancy-index += is an exact scatter-add.
    y = np.zeros((n + 1, D), np.float64)
    for e in range(N_CORES):
        # out is [128, CQ, D]: row of token slot t lives at [t%128, t//128]
        out_e = res.results[e]["out"].transpose(1, 0, 2).reshape(-1, D).astype(np.float64)
        g_e = res.results[e]["gout"].T.ravel()  # [C]: g for slot j*128+p
        out_e += g_e[:, None].astype(np.float64) * b2[e][None, :]
        y[scatter_ids[e]] += out_e
    return y[:n].astype(np.float32).reshape(B, S, D)
